# revision 1
# baseline (speedup 1.0000x reference)
"""BiLSTM POS tagger on 8 trn2 NeuronCores (Bass/Tile, SPMD).

Sharding: 2 direction groups (even cores = fwd, odd cores = bwd), hidden dim
split 4 ways within each group. Both LSTM layers fused into one scan over 512
steps; per step each core computes its 1024-row gate slice for both layers
(bf16 weight-stationary matmuls, N=64 moving = full batch), exchanges h-slices
with its group via one AllGather, and accumulates its contribution to the
output projection. Final combine via an 8-core ReduceScatter.

Self-contained: hardcodes all shapes; host side only casts/slices weights,
builds index tables, and reassembles the sharded output.
"""

import os
import sys

for _p in ("/opt/trn_rl_repo", "/root/.axon_site/_ro/trn_rl_repo"):
    if os.path.isdir(_p) and _p not in sys.path:
        sys.path.insert(0, _p)

import numpy as np
import ml_dtypes

from concourse import bacc, bass, mybir
import concourse.tile as tile
from concourse import bass_utils

B, S, V, E, H, O = 64, 512, 50000, 1024, 1024, 50
NSTEPS = int(os.environ.get("KERNEL_NSTEPS", S))  # dev-mode shrink
GRP = 4  # cores per direction group
KT = 8  # k tiles (1024/128)
MT = 8  # m tiles per core slice (1024/128)
NB = 4  # cc buffer ring
F32, BF16, I32 = mybir.dt.float32, mybir.dt.bfloat16, mybir.dt.int32

_prog_cache = {}


def _build_program(nsteps):
    nc = bacc.Bacc("TRN2", target_bir_lowering=False, debug=False, num_devices=8)

    # ---- I/O ----
    emb_d = nc.dram_tensor("emb", [V, E], BF16, kind="ExternalInput")
    w_d = nc.dram_tensor("wmats", [4, KT, MT, 128, 128], BF16, kind="ExternalInput")
    bias_d = nc.dram_tensor("biases", [128, 2, MT], F32, kind="ExternalInput")
    wout_d = nc.dram_tensor("wout", [KT, 128, O], BF16, kind="ExternalInput")
    bout_d = nc.dram_tensor("bout", [O, 1], F32, kind="ExternalInput")
    idx_d = nc.dram_tensor("idx", [B, S], I32, kind="ExternalInput")
    slot_d = nc.dram_tensor("slots", [O, S], I32, kind="ExternalInput")
    iden_d = nc.dram_tensor("iden", [128, 128], BF16, kind="ExternalInput")
    oshard_d = nc.dram_tensor("out_shard", [nsteps // 8, O, B], F32, kind="ExternalOutput")
    debug = os.environ.get("KERNEL_DEBUG", "0") == "1"
    if debug:
        dbgx_d = nc.dram_tensor("dbg_x", [128, KT, B], F32, kind="ExternalOutput")
        dbgh_d = nc.dram_tensor("dbg_h", [128, KT, B], F32, kind="ExternalOutput")

    # ---- SBUF persistents ----
    wsb = nc.alloc_sbuf_tensor("wsb", [128, 4, KT, MT, 128], BF16)
    bias_sb = nc.alloc_sbuf_tensor("bias_sb", [128, 2, MT], F32)
    wout_sb = nc.alloc_sbuf_tensor("wout_sb", [128, KT, O], BF16)
    bout_sb = nc.alloc_sbuf_tensor("bout_sb", [O, 1], F32)
    idx_sb = nc.alloc_sbuf_tensor("idx_sb", [B, S], I32)
    slot_sb = nc.alloc_sbuf_tensor("slot_sb", [O, S], I32)
    iden_sb = nc.alloc_sbuf_tensor("iden_sb", [128, 128], BF16)
    hf1 = [nc.alloc_sbuf_tensor(f"hf1_{p}", [128, KT, B], BF16) for p in range(2)]
    hf2 = [nc.alloc_sbuf_tensor(f"hf2_{p}", [128, KT, B], BF16) for p in range(2)]
    c1 = [nc.alloc_sbuf_tensor(f"c1_{p}", [128, 2, B], F32) for p in range(2)]
    c2 = [nc.alloc_sbuf_tensor(f"c2_{p}", [128, 2, B], F32) for p in range(2)]

    # ---- DRAM internals ----
    cc_in = [nc.dram_tensor(f"cc_in{i}", [128, 4 * B], BF16, kind="Internal") for i in range(NB)]
    cc_out = [
        nc.dram_tensor(f"cc_out{i}", [GRP, 128, 4 * B], BF16, kind="Internal")
        for i in range(NB)
    ]
    partial = nc.dram_tensor("partial", [nsteps * O, B], F32, kind="Internal")
    rs_out = nc.dram_tensor("rs_out", [nsteps * O // 8, B], F32, kind="Internal")

    AG_GROUPS = [[0, 2, 4, 6], [1, 3, 5, 7]]
    RS_GROUPS = [[0, 1, 2, 3, 4, 5, 6, 7]]

    with tile.TileContext(nc) as tc:
        # prologue: load constants
        for mi in range(4):
            nc.sync.dma_start(out=wsb[:, mi], in_=w_d[mi].transpose([2, 0, 1, 3]))
        nc.sync.dma_start(out=bias_sb[:], in_=bias_d[:])
        nc.sync.dma_start(out=wout_sb[:], in_=wout_d[:].transpose([1, 0, 2]))
        nc.sync.dma_start(out=bout_sb[:], in_=bout_d[:])
        nc.sync.dma_start(out=idx_sb[:], in_=idx_d[:])
        nc.sync.dma_start(out=slot_sb[:], in_=slot_d[:])
        nc.sync.dma_start(out=iden_sb[:], in_=iden_d[:])
        for p in range(2):
            nc.vector.memset(hf1[p][:], 0.0)
            nc.vector.memset(hf2[p][:], 0.0)
        nc.vector.memset(c1[1][:], 0.0)
        nc.vector.memset(c2[0][:], 0.0)

        from contextlib import ExitStack

        _stk = ExitStack()
        pool = _stk.enter_context(tc.tile_pool(name="sb", bufs=3))
        tmp_pool = _stk.enter_context(tc.tile_pool(name="tmp", bufs=6))
        xrow_pool = _stk.enter_context(tc.tile_pool(name="xrow", bufs=3))
        xt_pool = _stk.enter_context(tc.tile_pool(name="xt", bufs=3))
        pg_pool = _stk.enter_context(tc.tile_pool(name="pg", bufs=2, space="PSUM"))
        ptx_pool = _stk.enter_context(tc.tile_pool(name="ptx", bufs=2, space="PSUM"))
        po_pool = _stk.enter_context(tc.tile_pool(name="po", bufs=2, space="PSUM"))

        xT = {}

        def prepare_x_gather(t):
            xr = xrow_pool.tile([B, E], BF16, tag="xrow")
            nc.gpsimd.indirect_dma_start(
                out=xr[:],
                out_offset=None,
                in_=emb_d[:],
                in_offset=bass.IndirectOffsetOnAxis(ap=idx_sb[:, t : t + 1], axis=0),
            )
            return xr

        def prepare_x_transpose(t, xr):
            pt = ptx_pool.tile([128, KT, B], BF16, tag="ptx")
            for k in range(KT):
                nc.tensor.transpose(
                    out=pt[:, k, :], in_=xr[:, 128 * k : 128 * (k + 1)],
                    identity=iden_sb[0:B, 0:B],
                )
            xt = xt_pool.tile([128, KT, B], BF16, tag="xt")
            for k in range(KT):
                nc.vector.tensor_copy(out=xt[:, k, :], in_=pt[:, k, :])
            xT[t] = xt

        SIG = mybir.ActivationFunctionType.Sigmoid
        TANH = mybir.ActivationFunctionType.Tanh

        def layer_cell(lyr, pg, c_state, cur, prv, hdst):
            """Activations + cell update for one layer; writes h slice to hdst."""
            act = pool.tile([128, MT, B], F32, tag=f"act{lyr}")
            for m in range(MT):
                fn = TANH if m in (4, 5) else SIG
                nc.scalar.activation(
                    act[:, m, :], pg[:, m, :], fn,
                    bias=bias_sb[:, lyr, m : m + 1],
                )
            t1 = tmp_pool.tile([128, 2, B], F32, tag="t1")
            t2 = tmp_pool.tile([128, 2, B], F32, tag="t2")
            tch = tmp_pool.tile([128, 2, B], F32, tag="tch")
            nc.vector.tensor_mul(out=t1[:], in0=act[:, 2:4, :], in1=c_state[prv][:])
            nc.vector.tensor_mul(out=t2[:], in0=act[:, 0:2, :], in1=act[:, 4:6, :])
            nc.vector.tensor_add(out=c_state[cur][:], in0=t1[:], in1=t2[:])
            nc.scalar.activation(tch[:], c_state[cur][:], TANH)
            nc.vector.tensor_mul(out=hdst, in0=act[:, 6:8, :], in1=tch[:])

        xr_next = prepare_x_gather(0)
        prepare_x_transpose(0, xr_next)

        for t in range(nsteps + 2):
            cur, prv = t % 2, 1 - t % 2
            do_l1 = t < nsteps
            do_l2 = 1 <= t <= nsteps
            do_op = 2 <= t
            do_ag = t <= nsteps

            if do_l1 and t + 1 < nsteps:
                xr_next = prepare_x_gather(t + 1)

            hsl = pool.tile([128, 4, B], BF16, tag="hsl")

            # ---- PE: L1 matmuls ----
            if do_l1:
                pg1 = pg_pool.tile([128, MT, B], F32, tag="pg1")
                for m in range(MT):
                    for k in range(KT):
                        nc.tensor.matmul(
                            out=pg1[:, m, :], lhsT=wsb[:, 0, k, m, :],
                            rhs=xT[t][:, k, :], start=(k == 0), stop=False,
                        )
                    for k in range(KT):
                        nc.tensor.matmul(
                            out=pg1[:, m, :], lhsT=wsb[:, 1, k, m, :],
                            rhs=hf1[prv][:, k, :], start=False, stop=(k == KT - 1),
                        )
            # ---- PE: L2 matmuls ----
            if do_l2:
                pg2 = pg_pool.tile([128, MT, B], F32, tag="pg2")
                for m in range(MT):
                    for k in range(KT):
                        nc.tensor.matmul(
                            out=pg2[:, m, :], lhsT=wsb[:, 2, k, m, :],
                            rhs=hf1[prv][:, k, :], start=(k == 0), stop=False,
                        )
                    for k in range(KT):
                        nc.tensor.matmul(
                            out=pg2[:, m, :], lhsT=wsb[:, 3, k, m, :],
                            rhs=hf2[prv][:, k, :], start=False, stop=(k == KT - 1),
                        )
            # ---- activations + cell ----
            if debug and t == 0:
                dbg_pg1 = pool.tile([128, MT, B], F32, tag="dbgpg")
                nc.vector.tensor_copy(out=dbg_pg1[:], in_=pg1[:])
            if do_l1:
                layer_cell(0, pg1, c1, cur, prv, hsl[:, 0:2, :])
            else:
                nc.vector.memset(hsl[:, 0:2, :], 0.0)
            if do_l2:
                layer_cell(1, pg2, c2, cur, prv, hsl[:, 2:4, :])
            else:
                nc.vector.memset(hsl[:, 2:4, :], 0.0)

            # ---- output projection for step t-2 ----
            if do_op:
                s = t - 2
                po = po_pool.tile([O, B], F32, tag="po")
                for j in range(KT):
                    nc.tensor.matmul(
                        out=po[:], lhsT=wout_sb[:, j, :], rhs=hf2[prv][:, j, :],
                        start=(j == 0), stop=(j == KT - 1),
                    )
                outp = pool.tile([O, B], F32, tag="outp")
                nc.scalar.add(outp[:], po[:], bout_sb[:, 0:1])
                nc.gpsimd.indirect_dma_start(
                    out=partial[:],
                    out_offset=bass.IndirectOffsetOnAxis(
                        ap=slot_sb[:, s : s + 1], axis=0
                    ),
                    in_=outp[:],
                    in_offset=None,
                )

            # ---- h exchange (AllGather within direction group) ----
            if do_ag:
                nb = t % NB
                nc.sync.dma_start(
                    out=cc_in[nb][:], in_=hsl[:].rearrange("p a b -> p (a b)")
                )
                nc.gpsimd.collective_compute(
                    "AllGather", mybir.AluOpType.bypass, replica_groups=AG_GROUPS,
                    ins=[cc_in[nb][:]], outs=[cc_out[nb][:]],
                )
                nc.sync.dma_start(
                    out=hf1[cur][:].rearrange("p (r j) b -> p r j b", r=GRP),
                    in_=cc_out[nb][:, :, 0 : 2 * B].rearrange(
                        "r p (j b) -> p r j b", j=2
                    ),
                )
                nc.sync.dma_start(
                    out=hf2[cur][:].rearrange("p (r j) b -> p r j b", r=GRP),
                    in_=cc_out[nb][:, :, 2 * B : 4 * B].rearrange(
                        "r p (j b) -> p r j b", j=2
                    ),
                )

            if debug and t == 0:
                dbgx_sb = nc.alloc_sbuf_tensor("dbgx_sb", [128, KT, B], F32)
                dbgh_sb = nc.alloc_sbuf_tensor("dbgh_sb", [128, KT, B], F32)
                nc.vector.tensor_copy(out=dbgx_sb[:], in_=dbg_pg1[:])
                nc.vector.tensor_copy(out=dbgh_sb[:], in_=hf1[cur][:])
                nc.sync.dma_start(out=dbgx_d[:], in_=dbgx_sb[:])
                nc.sync.dma_start(out=dbgh_d[:], in_=dbgh_sb[:])

            # ---- x transpose for next step ----
            if do_l1 and t + 1 < nsteps:
                prepare_x_transpose(t + 1, xr_next)
            if t - 1 in xT:
                del xT[t - 1]

        # ---- final combine ----
        nc.gpsimd.collective_compute(
            "ReduceScatter", mybir.AluOpType.add, replica_groups=RS_GROUPS,
            ins=[partial[:]], outs=[rs_out[:]],
        )
        nrow = nsteps * O // 8
        bounce = nc.alloc_sbuf_tensor("bounce", [50, nrow // 50 * B], F32)
        nx = nrow // 50
        nc.sync.dma_start(
            out=bounce[:].rearrange("p (x b) -> p x b", x=nx),
            in_=rs_out[:].rearrange("(x p) b -> p x b", p=50),
        )
        nc.sync.dma_start(
            out=oshard_d[:].rearrange("s o b -> (s o) b").rearrange(
                "(x p) b -> p x b", p=50
            ),
            in_=bounce[:].rearrange("p (x b) -> p x b", x=nx),
        )

        _stk.close()

    nc.compile()
    return nc


def _host_prep(inputs, nsteps):
    src = np.asarray(inputs["src"])
    emb = np.asarray(inputs["embedding"], np.float32).astype(ml_dtypes.bfloat16)
    iden = np.eye(128, dtype=ml_dtypes.bfloat16)

    in_maps = []
    for c in range(8):
        d = c % 2  # 0 = fwd (even cores), 1 = bwd (odd cores)
        g = c // 2  # position within group
        if d == 0:
            wih = [np.asarray(inputs["Wih_fwd"][l], np.float32) for l in range(2)]
            whh = [np.asarray(inputs["Whh_fwd"][l], np.float32) for l in range(2)]
            bb = [np.asarray(inputs["b_fwd"][l], np.float32) for l in range(2)]
        else:
            wih = [np.asarray(inputs["Wih_bwd"][l], np.float32) for l in range(2)]
            whh = [np.asarray(inputs["Whh_bwd"][l], np.float32) for l in range(2)]
            bb = [np.asarray(inputs["b_bwd"][l], np.float32) for l in range(2)]

        rows = np.concatenate([np.arange(gate * H + 256 * g, gate * H + 256 * (g + 1))
                               for gate in range(4)])
        wmats = np.zeros((4, KT, MT, 128, 128), np.float32)
        for mi, mat in enumerate([wih[0], whh[0], wih[1], whh[1]]):
            sl = mat[rows].T  # lhsT [1024(k), 1024(m)]
            wmats[mi] = sl.reshape(KT, 128, MT, 128).transpose(0, 2, 1, 3)
        biases = np.stack([bb[0][rows], bb[1][rows]]).reshape(2, MT, 128).transpose(2, 0, 1)

        wout_full = np.asarray(inputs["Wout"], np.float32)  # [O, 2H]
        wd = wout_full[:, d * H : (d + 1) * H]  # [O, H]
        wout = np.zeros((KT, 128, O), np.float32)
        for j in (2 * g, 2 * g + 1):
            wout[j] = wd[:, 128 * j : 128 * (j + 1)].T
        bout = (np.asarray(inputs["bout"], np.float32).reshape(O, 1)
                if c == 0 else np.zeros((O, 1), np.float32))

        idx0 = src[:, :nsteps] if d == 0 else src[:, :nsteps][:, ::-1]
        idx = np.zeros((B, S), np.int32)
        idx[:, :nsteps] = idx0
        po = np.arange(O)
        ss = np.arange(nsteps)
        tmap = ss if d == 0 else (nsteps - 1 - ss)
        slots = np.zeros((O, S), np.int32)
        slots[:, :nsteps] = (tmap[None, :] * O + po[:, None]).astype(np.int32)

        in_maps.append({
            "emb": emb,
            "wmats": wmats.astype(ml_dtypes.bfloat16),
            "biases": np.ascontiguousarray(biases).astype(np.float32),
            "wout": wout.astype(ml_dtypes.bfloat16),
            "bout": bout,
            "idx": np.ascontiguousarray(idx),
            "slots": slots,
            "iden": iden,
        })
    return in_maps


def kernel(**inputs) -> np.ndarray:
    nsteps = NSTEPS
    if nsteps not in _prog_cache:
        _prog_cache[nsteps] = _build_program(nsteps)
    nc = _prog_cache[nsteps]
    in_maps = _host_prep(inputs, nsteps)
    res = bass_utils.run_bass_kernel_spmd(nc, in_maps, list(range(8)))
    shards = [res.results[c]["out_shard"] for c in range(8)]
    full = np.concatenate(shards, axis=0)  # [nsteps, O, B]
    return np.ascontiguousarray(full.transpose(2, 0, 1)).astype(np.float32)

